# revision 7
# baseline (speedup 1.0000x reference)
"""Multi-head causal attention (B=2, S=2048, D=1024, H=16) on 8 NeuronCores.

Sharding: data-parallel over batch (2 groups of 4 cores) x tensor-parallel over
heads (4 heads per core).  Each core projects q/k/v for its 4 heads, runs causal
flash-style attention, normalizes, then the 4 cores of a batch AllGather their
context (ctxT, [256,2048] each -> [1024,2048]) and each computes a 256-column
slice of the output projection.  Host assembles the 8 output slices.

All matmuls run in float32r (TF32-like, full PE speed at free-dim >= 256).
Softmax uses no max-subtraction: scaled scores are bounded (|s| < 4 for this
operator family), so exp is computed directly and the denominator is obtained
by augmenting V with a ones column (row 64 of the AV accumulation).
"""
import numpy as np

import concourse.bass as bass
import concourse.mybir as mybir
import concourse.tile as tile
from concourse.bass_utils import run_bass_kernel_spmd

# ---------------------------------------------------------------- constants
B, S, D, H, HD = 2, 2048, 1024, 16, 64
NCORES = 8
HLOC = 4              # heads per core
OLOC = HLOC * HD      # 256 local qkv features
P = 128               # partitions
SBK = 512             # big seq block (moving free dim)
NSB = S // SBK        # 4
NFC = D // P          # 8 feature chunks
NKC = S // P          # 16 key chunks
F32 = mybir.dt.float32
F32R = mybir.dt.float32r

_CACHE = {}

# ------------------------------------------------------------- wait legalizer
_wl_counter = [0]


def _legalize_waits(nc):
    """This walrus build allows only ONE inline sync-wait per instruction.
    Move extra waits onto NoOps inserted before, on the same engine stream."""
    for bb in nc.main_func.blocks:
        insts = bb.instructions
        new_list = []
        changed = False
        for inst in insts:
            si = getattr(inst, "sync_info", None)
            waits = list(si.on_wait) if si is not None and si.on_wait else []
            if len(waits) > 1:
                for w in waits[1:]:
                    _wl_counter[0] += 1
                    noop = mybir.InstNoOp(
                        name=f"waitsplit-{_wl_counter[0]}",
                        sync_info=mybir.SyncInfo(on_wait=[w], on_update=[]),
                        bass_nofuse=True,
                        engine=inst.engine,
                    )
                    nc.register_instruction(noop, overwrite=True)
                    new_list.append(noop)
                si.on_wait = waits[:1]
                changed = True
            new_list.append(inst)
        if changed:
            bb.instructions[:] = new_list
    return nc


# ---------------------------------------------------------------- the kernel
def _build_nc():
    nc = bass.Bass(num_devices=NCORES)

    xt = nc.dram_tensor("xt", [D, S], F32R, kind="ExternalInput")
    wq = nc.dram_tensor("wq", [D, OLOC], F32R, kind="ExternalInput")
    wk = nc.dram_tensor("wk", [D, OLOC], F32R, kind="ExternalInput")
    wv = nc.dram_tensor("wv", [D, OLOC], F32R, kind="ExternalInput")
    wo = nc.dram_tensor("wo", [D, OLOC], F32R, kind="ExternalInput")
    bo = nc.dram_tensor("bo", [P, 2], F32, kind="ExternalInput")
    masks = nc.dram_tensor("masks", [P, 4 * SBK], F32R, kind="ExternalInput")
    ones_in = nc.dram_tensor("ones_in", [P, 68], F32R, kind="ExternalInput")
    outT = nc.dram_tensor("outT", [OLOC, S], F32, kind="ExternalOutput")

    Exp = mybir.ActivationFunctionType.Exp
    Ident = mybir.ActivationFunctionType.Identity

    with tile.TileContext(nc) as tc:
        with (
            tc.tile_pool(name="const", bufs=1) as constp,
            tc.tile_pool(name="wsb", bufs=1) as wsb,
            tc.tile_pool(name="qk", bufs=1) as qkp,
            tc.tile_pool(name="vtp", bufs=1) as vtp,
            tc.tile_pool(name="ctx", bufs=1) as ctxp_pool,
            tc.tile_pool(name="dram", bufs=1, space="DRAM") as dramp,
        ):
            # ---- constants / weights resident in SBUF
            masks_sb = constp.tile([P, 4 * SBK], F32R)
            nc.sync.dma_start(masks_sb[:], masks[:])
            bo_sb = constp.tile([P, 2], F32)
            nc.sync.dma_start(bo_sb[:], bo[:])

            wq_sb = wsb.tile([P, NFC * OLOC], F32R, tag="wq")
            wk_sb = wsb.tile([P, NFC * OLOC], F32R, tag="wk")
            wv_sb = wsb.tile([P, NFC * OLOC], F32R, tag="wv")
            wo_sb = constp.tile([P, NFC * OLOC], F32R)
            for t_sb, t_dr in ((wq_sb, wq), (wk_sb, wk), (wv_sb, wv), (wo_sb, wo)):
                for fc in range(NFC):
                    nc.sync.dma_start(
                        t_sb[:, fc * OLOC:(fc + 1) * OLOC],
                        t_dr[fc * P:(fc + 1) * P, :],
                    )

            # ---- persistent activations
            # qT/kT: per o-block (2) x seq-block (4), [128, 512] (o on partitions)
            qT = [[qkp.tile([P, SBK], F32R, name=f"qT{ob}{sb}", tag=f"q{ob}{sb}")
                   for sb in range(NSB)] for ob in range(2)]
            kT = [[qkp.tile([P, SBK], F32R, name=f"kT{ob}{sb}", tag=f"k{ob}{sb}")
                   for sb in range(NSB)] for ob in range(2)]
            # v natural per key-chunk: [128, 4*65]; head h at cols 65h..65h+63,
            # ones column at 65h+64 (gives softmax denominator in AV row 64).
            vt = [vtp.tile([P, HLOC * (HD + 1)], F32R, name=f"vt{kc}", tag=f"v{kc}")
                  for kc in range(NKC)]
            for kc in range(NKC):
                v3 = vt[kc].rearrange("p (h x) -> p h x", h=HLOC)
                nc.sync.dma_start(v3[:, :, HD:HD + 1], ones_in[:, 64:68])
            # ctxT (normalized) per o-block: [128, 2048]
            ctxT = [ctxp_pool.tile([P, S], F32R, name=f"ctxT{ob}", tag=f"c{ob}")
                    for ob in range(2)]

            # ================= phase A: q/k/v projections =================
            with (
                tc.tile_pool(name="xc", bufs=12) as xcp,
                tc.tile_pool(name="psA", bufs=4, space="PSUM") as psA,
                tc.tile_pool(name="psV", bufs=2, space="PSUM") as psV,
            ):
                for sb in range(NSB):
                    xch = []
                    for fc in range(NFC):
                        t = xcp.tile([P, SBK], F32R, tag="x")
                        nc.sync.dma_start(
                            t[:], xt[fc * P:(fc + 1) * P, sb * SBK:(sb + 1) * SBK])
                        xch.append(t)
                    for (name, w_sb, dst) in (("q", wq_sb, qT), ("k", wk_sb, kT)):
                        for ob in range(2):
                            ps = psA.tile([P, SBK], F32, tag="pqk")
                            for fc in range(NFC):
                                lhsT = w_sb[:, fc * OLOC + ob * P:
                                            fc * OLOC + (ob + 1) * P]
                                nc.tensor.matmul(ps[:], lhsT, xch[fc][:],
                                                 start=(fc == 0), stop=(fc == NFC - 1))
                            nc.vector.tensor_copy(dst[ob][sb][:], ps[:])
                    for sc in range(4):
                        kc = 4 * sb + sc
                        ps = psV.tile([P, OLOC], F32, tag="pv")
                        for fc in range(NFC):
                            lhsT = xch[fc][:, sc * P:(sc + 1) * P]
                            nc.tensor.matmul(ps[:], lhsT,
                                             wv_sb[:, fc * OLOC:(fc + 1) * OLOC],
                                             start=(fc == 0), stop=(fc == NFC - 1))
                        v3 = vt[kc].rearrange("p (h x) -> p h x", h=HLOC)
                        p3 = ps.rearrange("p (h x) -> p h x", h=HLOC)
                        nc.vector.tensor_copy(v3[:, :, 0:HD], p3[:])

            # ================= phase B: causal attention =================
            with (
                tc.tile_pool(name="exp", bufs=6) as expp,
                tc.tile_pool(name="sm", bufs=4) as smp,
                tc.tile_pool(name="psS", bufs=3, space="PSUM") as psS,
                tc.tile_pool(name="psC", bufs=2, space="PSUM") as psC,
                tc.tile_pool(name="psB", bufs=2, space="PSUM") as psB,
            ):
                ones_l = smp.tile([1, HD], F32R, name="ones_l", tag="ones")
                nc.sync.dma_start(ones_l[:], ones_in[0:1, 0:HD])
                for qb in range(NSB):
                    for ob in range(2):
                        for j in range(2):
                            h = 2 * ob + j
                            nkb = 4 * qb + 4
                            ctx_ps = psC.tile([HD + 1, SBK], F32, tag="ctx")
                            for kb in range(nkb):
                                sp = psS.tile([P, SBK], F32, tag="sc")
                                nc.tensor.matmul(
                                    sp[:],
                                    kT[ob][kb // 4][j * HD:(j + 1) * HD,
                                                    (kb % 4) * P:(kb % 4 + 1) * P],
                                    qT[ob][qb][j * HD:(j + 1) * HD, :],
                                    start=True, stop=True,
                                    tile_position=(j * HD, 0),
                                )
                                et = expp.tile([P, SBK], F32R, tag="e")
                                nc.scalar.activation(et[:], sp[:], Exp, scale=0.125)
                                t = kb - 4 * qb
                                if t >= 0:  # diagonal block: zero out k > q
                                    nc.vector.tensor_tensor(
                                        et[:], et[:],
                                        masks_sb[:, t * SBK:(t + 1) * SBK],
                                        mybir.AluOpType.mult,
                                    )
                                nc.tensor.matmul(
                                    ctx_ps[:],
                                    vt[kb][:, h * (HD + 1):(h + 1) * (HD + 1)],
                                    et[:],
                                    start=(kb == 0), stop=(kb == nkb - 1),
                                )
                            # normalize: row HD of ctx_ps is the denominator
                            rc = smp.tile([1, SBK], F32, tag="rc")
                            nc.vector.reciprocal(rc[:], ctx_ps[HD:HD + 1, :])
                            rcr = smp.tile([1, SBK], F32R, tag="rcr")
                            nc.vector.tensor_copy(rcr[:], rc[:])
                            bc_ps = psB.tile([HD, SBK], F32, tag="bcp")
                            nc.tensor.matmul(bc_ps[:], ones_l[:], rcr[:],
                                             start=True, stop=True)
                            bc = smp.tile([HD, SBK], F32, tag="bc")
                            nc.vector.tensor_copy(bc[:], bc_ps[:])
                            nc.vector.tensor_tensor(
                                ctxT[ob][j * HD:(j + 1) * HD,
                                         qb * SBK:(qb + 1) * SBK],
                                ctx_ps[0:HD, :], bc[:],
                                mybir.AluOpType.mult,
                            )

            # ============ phase C: AllGather ctx + output projection ============
            cin = dramp.tile([OLOC, S], F32R)
            cout = dramp.tile([HLOC * OLOC, S], F32R)
            nc.sync.dma_start(cin[0:P, :], ctxT[0][:])
            nc.sync.dma_start(cin[P:OLOC, :], ctxT[1][:])
            nc.gpsimd.collective_compute(
                "AllGather",
                mybir.AluOpType.bypass,
                replica_groups=[[0, 1, 2, 3], [4, 5, 6, 7]],
                ins=[cin.opt()],
                outs=[cout.opt()],
            )
            with (
                tc.tile_pool(name="gth", bufs=1) as gthp,
                tc.tile_pool(name="osb", bufs=4) as osbp,
                tc.tile_pool(name="psO", bufs=4, space="PSUM") as psO,
            ):
                g = [gthp.tile([P, S], F32R, name=f"g{oc}", tag=f"g{oc}") for oc in range(NFC)]
                for oc in range(NFC):
                    nc.sync.dma_start(g[oc][:], cout[oc * P:(oc + 1) * P, :])
                for sb in range(NSB):
                    for cb in range(2):
                        ps = psO.tile([P, SBK], F32, tag="po")
                        for oc in range(NFC):
                            lhsT = wo_sb[:, oc * OLOC + cb * P:
                                         oc * OLOC + (cb + 1) * P]
                            nc.tensor.matmul(ps[:], lhsT,
                                             g[oc][:, sb * SBK:(sb + 1) * SBK],
                                             start=(oc == 0), stop=(oc == NFC - 1))
                        ot = osbp.tile([P, SBK], F32, tag="ot")
                        nc.scalar.activation(ot[:], ps[:], Ident,
                                             bias=bo_sb[:, cb:cb + 1], scale=1.0)
                        nc.sync.dma_start(
                            outT[cb * P:(cb + 1) * P, sb * SBK:(sb + 1) * SBK],
                            ot[:])

    _legalize_waits(nc)
    return nc


def _get_nc():
    if "nc" not in _CACHE:
        _CACHE["nc"] = _build_nc()
    return _CACHE["nc"]


LAST_RESULTS = None  # BassKernelResults of the most recent run (for profiling)


def kernel(x, Wq, Wk, Wv, Wo, bo):
    global LAST_RESULTS
    x = np.ascontiguousarray(np.asarray(x, dtype=np.float32))
    Wq = np.asarray(Wq, dtype=np.float32)
    Wk = np.asarray(Wk, dtype=np.float32)
    Wv = np.asarray(Wv, dtype=np.float32)
    Wo = np.asarray(Wo, dtype=np.float32)
    bo = np.asarray(bo, dtype=np.float32)

    # causal masks for the 4 diagonal 128x512 blocks: valid iff qi >= ki + 128*t
    ki = np.arange(P)[:, None]
    qi = np.arange(SBK)[None, :]
    masks = np.concatenate(
        [(qi >= ki + P * t).astype(np.float32) for t in range(4)], axis=1)

    in_maps = []
    for c in range(NCORES):
        b, g = divmod(c, HLOC)
        sl = slice(g * OLOC, (g + 1) * OLOC)
        in_maps.append({
            "xt": np.ascontiguousarray(x[b].T),
            "wq": np.ascontiguousarray(Wq[sl, :].T),
            "wk": np.ascontiguousarray(Wk[sl, :].T),
            "wv": np.ascontiguousarray(Wv[sl, :].T),
            "wo": np.ascontiguousarray(Wo[sl, :].T),
            "bo": np.ascontiguousarray(bo[sl].reshape(2, P).T),
            "masks": masks,
            "ones_in": np.ones((P, 68), dtype=np.float32),
        })

    nc = _get_nc()
    LAST_RESULTS = run_bass_kernel_spmd(nc, in_maps, core_ids=list(range(NCORES)))

    out = np.empty((B, S, D), dtype=np.float32)
    for c in range(NCORES):
        b, g = divmod(c, HLOC)
        out[b, :, g * OLOC:(g + 1) * OLOC] = LAST_RESULTS.results[c]["outT"].T
    return out


# revision 11
# speedup vs baseline: 1.2954x; 1.2954x over previous
"""Multi-head causal attention (B=2, S=2048, D=1024, H=16) on 8 NeuronCores.

Sharding: data-parallel over batch (2 groups of 4 cores) x tensor-parallel over
heads (4 heads per core).  Each core projects q/k/v for its 4 heads, runs causal
flash-style attention, normalizes, then the 4 cores of a batch AllGather their
context (ctxT, [256,2048] each -> [1024,2048]) and each computes a 256-column
slice of the output projection.  Host assembles the 8 output slices.

All matmuls run in float32r (TF32-like, full PE speed at free-dim >= 256).
Softmax uses no max-subtraction: scaled scores are bounded (|s| < 4 for this
operator family), so exp is computed directly and the denominator is obtained
by augmenting V with a ones column (row 64 of the AV accumulation).
"""
import numpy as np

import concourse.bass as bass
import concourse.mybir as mybir
import concourse.tile as tile
from concourse.bass_utils import run_bass_kernel_spmd

# ---------------------------------------------------------------- constants
B, S, D, H, HD = 2, 2048, 1024, 16, 64
NCORES = 8
HLOC = 4              # heads per core
OLOC = HLOC * HD      # 256 local qkv features
P = 128               # partitions
SBK = 512             # big seq block (moving free dim)
NSB = S // SBK        # 4
NFC = D // P          # 8 feature chunks
NKC = S // P          # 16 key chunks
F32 = mybir.dt.float32
F32R = mybir.dt.float32r

_CACHE = {}

# ------------------------------------------------------------- wait legalizer
_wl_counter = [0]


def _legalize_waits(nc):
    """This walrus build allows only ONE inline sync-wait per instruction.
    Move extra waits onto NoOps inserted before, on the same engine stream."""
    for bb in nc.main_func.blocks:
        insts = bb.instructions
        new_list = []
        changed = False
        for inst in insts:
            si = getattr(inst, "sync_info", None)
            waits = list(si.on_wait) if si is not None and si.on_wait else []
            if len(waits) > 1:
                for w in waits[1:]:
                    _wl_counter[0] += 1
                    noop = mybir.InstNoOp(
                        name=f"waitsplit-{_wl_counter[0]}",
                        sync_info=mybir.SyncInfo(on_wait=[w], on_update=[]),
                        bass_nofuse=True,
                        engine=inst.engine,
                    )
                    nc.register_instruction(noop, overwrite=True)
                    new_list.append(noop)
                si.on_wait = waits[:1]
                changed = True
            new_list.append(inst)
        if changed:
            bb.instructions[:] = new_list
    return nc


# ---------------------------------------------------------------- the kernel
def _build_nc():
    nc = bass.Bass(num_devices=NCORES)

    xt = nc.dram_tensor("xt", [D, S], F32R, kind="ExternalInput")
    wq = nc.dram_tensor("wq", [D, OLOC], F32R, kind="ExternalInput")
    wk = nc.dram_tensor("wk", [D, OLOC], F32R, kind="ExternalInput")
    wv = nc.dram_tensor("wv", [D, OLOC], F32R, kind="ExternalInput")
    wo = nc.dram_tensor("wo", [D, OLOC], F32R, kind="ExternalInput")
    bo = nc.dram_tensor("bo", [P, 2], F32, kind="ExternalInput")
    masks = nc.dram_tensor("masks", [P, 4 * SBK], F32R, kind="ExternalInput")
    ones_in = nc.dram_tensor("ones_in", [P, 68], F32R, kind="ExternalInput")
    outT = nc.dram_tensor("outT", [OLOC, S], F32, kind="ExternalOutput")

    Exp = mybir.ActivationFunctionType.Exp
    Ident = mybir.ActivationFunctionType.Identity

    with tile.TileContext(nc) as tc:
        with (
            tc.tile_pool(name="const", bufs=1) as constp,
            tc.tile_pool(name="wsb", bufs=1) as wsb,
            tc.tile_pool(name="qk", bufs=1) as qkp,
            tc.tile_pool(name="vtp", bufs=1) as vtp,
            tc.tile_pool(name="ctx", bufs=1) as ctxp_pool,
            tc.tile_pool(name="dram", bufs=1, space="DRAM") as dramp,
        ):
            # ---- constants / weights resident in SBUF
            masks_sb = constp.tile([P, 4 * SBK], F32R)
            nc.sync.dma_start(masks_sb[:], masks[:])
            bo_sb = constp.tile([P, 2], F32)
            nc.sync.dma_start(bo_sb[:], bo[:])

            wq_sb = wsb.tile([P, NFC * OLOC], F32R, tag="wq")
            wk_sb = wsb.tile([P, NFC * OLOC], F32R, tag="wk")
            wv_sb = wsb.tile([P, NFC * OLOC], F32R, tag="wv")
            wo_sb = constp.tile([P, NFC * OLOC], F32R)
            for t_sb, t_dr in ((wq_sb, wq), (wk_sb, wk), (wv_sb, wv), (wo_sb, wo)):
                for fc in range(NFC):
                    nc.sync.dma_start(
                        t_sb[:, fc * OLOC:(fc + 1) * OLOC],
                        t_dr[fc * P:(fc + 1) * P, :],
                    )

            # ---- persistent activations
            # qT/kT: per o-block (2) x seq-block (4), [128, 512] (o on partitions)
            qT = [[qkp.tile([P, SBK], F32R, name=f"qT{ob}{sb}", tag=f"q{ob}{sb}")
                   for sb in range(NSB)] for ob in range(2)]
            kT = [[qkp.tile([P, SBK], F32R, name=f"kT{ob}{sb}", tag=f"k{ob}{sb}")
                   for sb in range(NSB)] for ob in range(2)]
            # v natural per key-chunk: [128, 4*65]; head h at cols 65h..65h+63,
            # ones column at 65h+64 (gives softmax denominator in AV row 64).
            vt = [vtp.tile([P, HLOC * (HD + 1)], F32R, name=f"vt{kc}", tag=f"v{kc}")
                  for kc in range(NKC)]
            for kc in range(NKC):
                v3 = vt[kc].rearrange("p (h x) -> p h x", h=HLOC)
                nc.sync.dma_start(v3[:, :, HD:HD + 1], ones_in[:, 64:68])
            # ctxT (normalized) per o-block x q-chunk: [128, 512]
            ctxq = [[ctxp_pool.tile([P, SBK], F32R, name=f"ctxq{ob}{qb}",
                                    tag=f"c{ob}{qb}") for qb in range(NSB)]
                    for ob in range(2)]

            # ================= phase A: q/k/v projections =================
            with (
                tc.tile_pool(name="xc", bufs=12) as xcp,
                tc.tile_pool(name="psA", bufs=4, space="PSUM") as psA,
                tc.tile_pool(name="psV", bufs=2, space="PSUM") as psV,
            ):
                for sb in range(NSB):
                    xch = []
                    for fc in range(NFC):
                        t = xcp.tile([P, SBK], F32R, tag="x")
                        nc.sync.dma_start(
                            t[:], xt[fc * P:(fc + 1) * P, sb * SBK:(sb + 1) * SBK])
                        xch.append(t)
                    for (name, w_sb, dst) in (("q", wq_sb, qT), ("k", wk_sb, kT)):
                        for ob in range(2):
                            ps = psA.tile([P, SBK], F32, tag="pqk")
                            for fc in range(NFC):
                                lhsT = w_sb[:, fc * OLOC + ob * P:
                                            fc * OLOC + (ob + 1) * P]
                                nc.tensor.matmul(ps[:], lhsT, xch[fc][:],
                                                 start=(fc == 0), stop=(fc == NFC - 1))
                            nc.vector.tensor_copy(dst[ob][sb][:], ps[:])
                    for sc in range(4):
                        kc = 4 * sb + sc
                        ps = psV.tile([P, OLOC], F32, tag="pv")
                        for fc in range(NFC):
                            lhsT = xch[fc][:, sc * P:(sc + 1) * P]
                            nc.tensor.matmul(ps[:], lhsT,
                                             wv_sb[:, fc * OLOC:(fc + 1) * OLOC],
                                             start=(fc == 0), stop=(fc == NFC - 1))
                        v3 = vt[kc].rearrange("p (h x) -> p h x", h=HLOC)
                        p3 = ps.rearrange("p (h x) -> p h x", h=HLOC)
                        nc.vector.tensor_copy(v3[:, :, 0:HD], p3[:])

            # ========== phase B+C: attention, chunked AllGather, out-proj ==========
            cin_q = [dramp.tile([OLOC, SBK], F32R, name=f"cin{qb}", tag=f"cin{qb}")
                     for qb in range(NSB)]
            cout_q = [dramp.tile([HLOC * OLOC, SBK], F32R, name=f"cout{qb}",
                                 tag=f"cout{qb}") for qb in range(NSB)]
            with (
                tc.tile_pool(name="exp", bufs=6) as expp,
                tc.tile_pool(name="sm", bufs=4) as smp,
                tc.tile_pool(name="gth", bufs=1) as gthp,
                tc.tile_pool(name="osb", bufs=4) as osbp,
                tc.tile_pool(name="psS", bufs=3, space="PSUM") as psS,
                tc.tile_pool(name="psC", bufs=2, space="PSUM") as psC,
                tc.tile_pool(name="psB", bufs=1, space="PSUM") as psB,
                tc.tile_pool(name="psO", bufs=2, space="PSUM") as psO,
            ):
                ones_l = smp.tile([1, HD], F32R, name="ones_l", tag="ones")
                nc.sync.dma_start(ones_l[:], ones_in[0:1, 0:HD])
                for qb in range(NSB):
                    nkb = 4 * qb + 4
                    # head h's denominator parked at partition 32h (alignment)
                    den = smp.tile([P, SBK], F32, name=f"den{qb}", tag="den")
                    rec = smp.tile([P, SBK], F32, name=f"rec{qb}", tag="rec")
                    ctx_ps_h = []
                    for ob in range(2):
                        for j in range(2):
                            h = 2 * ob + j
                            ctx_ps = psC.tile([HD + 1, SBK], F32, tag="ctx")
                            for kb in range(nkb):
                                t = kb - 4 * qb
                                off = 128 * t if t > 0 else 0
                                sp = psS.tile([P, SBK], F32, tag="sc")
                                nc.tensor.matmul(
                                    sp[:, off:],
                                    kT[ob][kb // 4][j * HD:(j + 1) * HD,
                                                    (kb % 4) * P:(kb % 4 + 1) * P],
                                    qT[ob][qb][j * HD:(j + 1) * HD, off:],
                                    start=True, stop=True,
                                    tile_position=(j * HD, 0),
                                )
                                et = expp.tile([P, SBK], F32R, tag="e")
                                nc.scalar.activation(et[:, off:], sp[:, off:],
                                                     Exp, scale=0.125)
                                if t >= 0:  # diagonal block: zero out k > q
                                    nc.vector.tensor_tensor(
                                        et[:, off:], et[:, off:],
                                        masks_sb[:, t * SBK + off:(t + 1) * SBK],
                                        mybir.AluOpType.mult,
                                    )
                                nc.tensor.matmul(
                                    ctx_ps[:, off:],
                                    vt[kb][:, h * (HD + 1):(h + 1) * (HD + 1)],
                                    et[:, off:],
                                    start=(kb == 0), stop=(kb == nkb - 1),
                                )
                            # stash denominator row (partition HD -> partition h)
                            nc.vector.tensor_copy(den[32 * h:32 * h + 1, :],
                                                  ctx_ps[HD:HD + 1, :])
                            cu = smp.tile([HD, SBK], F32, name=f"cu{qb}{h}",
                                          tag=f"cu{h}")
                            nc.scalar.copy(cu[:], ctx_ps[0:HD, :])
                            ctx_ps_h.append(cu)
                    # batched reciprocal for the 4 heads of this q-chunk
                    nc.vector.reciprocal(rec[:], den[:])
                    for ob in range(2):
                        for j in range(2):
                            h = 2 * ob + j
                            rcr = smp.tile([1, SBK], F32R, tag="rcr")
                            nc.vector.tensor_copy(rcr[:], rec[32 * h:32 * h + 1, :])
                            bc_ps = psB.tile([HD, SBK], F32, tag="bcp")
                            nc.tensor.matmul(bc_ps[:], ones_l[:], rcr[:],
                                             start=True, stop=True)
                            bc = smp.tile([HD, SBK], F32, tag="bc")
                            nc.vector.tensor_copy(bc[:], bc_ps[:])
                            nc.vector.tensor_tensor(
                                ctxq[ob][qb][j * HD:(j + 1) * HD, :],
                                ctx_ps_h[h][:], bc[:],
                                mybir.AluOpType.mult,
                            )
                    # ---- AllGather this q-chunk across the 4 cores of the batch
                    nc.sync.dma_start(cin_q[qb][0:P, :], ctxq[0][qb][:])
                    nc.sync.dma_start(cin_q[qb][P:OLOC, :], ctxq[1][qb][:])
                    nc.gpsimd.collective_compute(
                        "AllGather",
                        mybir.AluOpType.bypass,
                        replica_groups=[[0, 1, 2, 3], [4, 5, 6, 7]],
                        ins=[cin_q[qb].opt()],
                        outs=[cout_q[qb].opt()],
                    )
                    g = [gthp.tile([P, SBK], F32R, name=f"g{qb}{oc}",
                                   tag=f"g{oc}") for oc in range(NFC)]
                    for oc in range(NFC):
                        nc.sync.dma_start(g[oc][:],
                                          cout_q[qb][oc * P:(oc + 1) * P, :])
                    for cb in range(2):
                        ps = psO.tile([P, SBK], F32, tag="po")
                        for oc in range(NFC):
                            lhsT = wo_sb[:, oc * OLOC + cb * P:
                                         oc * OLOC + (cb + 1) * P]
                            nc.tensor.matmul(ps[:], lhsT, g[oc][:],
                                             start=(oc == 0), stop=(oc == NFC - 1))
                        ot = osbp.tile([P, SBK], F32, tag="ot")
                        nc.scalar.activation(ot[:], ps[:], Ident,
                                             bias=bo_sb[:, cb:cb + 1], scale=1.0)
                        nc.sync.dma_start(
                            outT[cb * P:(cb + 1) * P, qb * SBK:(qb + 1) * SBK],
                            ot[:])

    _legalize_waits(nc)
    return nc
def _get_nc():
    if "nc" not in _CACHE:
        _CACHE["nc"] = _build_nc()
    return _CACHE["nc"]


LAST_RESULTS = None  # BassKernelResults of the most recent run (for profiling)


def kernel(x, Wq, Wk, Wv, Wo, bo):
    global LAST_RESULTS
    x = np.ascontiguousarray(np.asarray(x, dtype=np.float32))
    Wq = np.asarray(Wq, dtype=np.float32)
    Wk = np.asarray(Wk, dtype=np.float32)
    Wv = np.asarray(Wv, dtype=np.float32)
    Wo = np.asarray(Wo, dtype=np.float32)
    bo = np.asarray(bo, dtype=np.float32)

    # causal masks for the 4 diagonal 128x512 blocks: valid iff qi >= ki + 128*t
    ki = np.arange(P)[:, None]
    qi = np.arange(SBK)[None, :]
    masks = np.concatenate(
        [(qi >= ki + P * t).astype(np.float32) for t in range(4)], axis=1)

    in_maps = []
    for c in range(NCORES):
        b, g = divmod(c, HLOC)
        sl = slice(g * OLOC, (g + 1) * OLOC)
        in_maps.append({
            "xt": np.ascontiguousarray(x[b].T),
            "wq": np.ascontiguousarray(Wq[sl, :].T),
            "wk": np.ascontiguousarray(Wk[sl, :].T),
            "wv": np.ascontiguousarray(Wv[sl, :].T),
            "wo": np.ascontiguousarray(Wo[sl, :].T),
            "bo": np.ascontiguousarray(bo[sl].reshape(2, P).T),
            "masks": masks,
            "ones_in": np.ones((P, 68), dtype=np.float32),
        })

    nc = _get_nc()
    LAST_RESULTS = run_bass_kernel_spmd(nc, in_maps, core_ids=list(range(NCORES)))

    out = np.empty((B, S, D), dtype=np.float32)
    for c in range(NCORES):
        b, g = divmod(c, HLOC)
        out[b, :, g * OLOC:(g + 1) * OLOC] = LAST_RESULTS.results[c]["outT"].T
    return out


# revision 12
# speedup vs baseline: 1.4158x; 1.0930x over previous
"""Multi-head causal attention (B=2, S=2048, D=1024, H=16) on 8 NeuronCores.

Sharding: data-parallel over batch (2 groups of 4 cores) x tensor-parallel over
heads (4 heads per core).  Each core projects q/k/v for its 4 heads, runs causal
flash-style attention, normalizes, then the 4 cores of a batch AllGather their
context (ctxT, [256,2048] each -> [1024,2048]) and each computes a 256-column
slice of the output projection.  Host assembles the 8 output slices.

All matmuls run in float32r (TF32-like, full PE speed at free-dim >= 256).
Softmax uses no max-subtraction: scaled scores are bounded (|s| < 4 for this
operator family), so exp is computed directly and the denominator is obtained
by augmenting V with a ones column (row 64 of the AV accumulation).
"""
import numpy as np

import concourse.bass as bass
import concourse.mybir as mybir
import concourse.tile as tile
from concourse.bass_utils import run_bass_kernel_spmd

# ---------------------------------------------------------------- constants
B, S, D, H, HD = 2, 2048, 1024, 16, 64
NCORES = 8
HLOC = 4              # heads per core
OLOC = HLOC * HD      # 256 local qkv features
P = 128               # partitions
SBK = 512             # big seq block (moving free dim)
NSB = S // SBK        # 4
NFC = D // P          # 8 feature chunks
NKC = S // P          # 16 key chunks
F32 = mybir.dt.float32
F32R = mybir.dt.float32r

_CACHE = {}

# ------------------------------------------------------------- wait legalizer
_wl_counter = [0]


def _legalize_waits(nc):
    """This walrus build allows only ONE inline sync-wait per instruction.
    Move extra waits onto NoOps inserted before, on the same engine stream."""
    for bb in nc.main_func.blocks:
        insts = bb.instructions
        new_list = []
        changed = False
        for inst in insts:
            si = getattr(inst, "sync_info", None)
            waits = list(si.on_wait) if si is not None and si.on_wait else []
            if len(waits) > 1:
                for w in waits[1:]:
                    _wl_counter[0] += 1
                    noop = mybir.InstNoOp(
                        name=f"waitsplit-{_wl_counter[0]}",
                        sync_info=mybir.SyncInfo(on_wait=[w], on_update=[]),
                        bass_nofuse=True,
                        engine=inst.engine,
                    )
                    nc.register_instruction(noop, overwrite=True)
                    new_list.append(noop)
                si.on_wait = waits[:1]
                changed = True
            new_list.append(inst)
        if changed:
            bb.instructions[:] = new_list
    return nc


# ---------------------------------------------------------------- the kernel
def _build_nc():
    nc = bass.Bass(num_devices=NCORES)

    xt = nc.dram_tensor("xt", [D, S], F32R, kind="ExternalInput")
    wq = nc.dram_tensor("wq", [D, OLOC], F32R, kind="ExternalInput")
    wk = nc.dram_tensor("wk", [D, OLOC], F32R, kind="ExternalInput")
    wv = nc.dram_tensor("wv", [D, OLOC], F32R, kind="ExternalInput")
    wo = nc.dram_tensor("wo", [D, OLOC], F32R, kind="ExternalInput")
    bo = nc.dram_tensor("bo", [P, 2], F32, kind="ExternalInput")
    masks = nc.dram_tensor("masks", [P, 4 * SBK], F32R, kind="ExternalInput")
    ones_in = nc.dram_tensor("ones_in", [P, 68], F32R, kind="ExternalInput")
    outT = nc.dram_tensor("outT", [OLOC, S], F32, kind="ExternalOutput")

    Exp = mybir.ActivationFunctionType.Exp
    Ident = mybir.ActivationFunctionType.Identity

    with tile.TileContext(nc) as tc:
        with (
            tc.tile_pool(name="const", bufs=1) as constp,
            tc.tile_pool(name="wsb", bufs=1) as wsb,
            tc.tile_pool(name="qk", bufs=1) as qkp,
            tc.tile_pool(name="vtp", bufs=1) as vtp,
            tc.tile_pool(name="ctx", bufs=1) as ctxp_pool,
            tc.tile_pool(name="dram", bufs=1, space="DRAM") as dramp,
        ):
            # ---- constants / weights resident in SBUF
            masks_sb = constp.tile([P, 4 * SBK], F32R)
            nc.sync.dma_start(masks_sb[:], masks[:])
            bo_sb = constp.tile([P, 2], F32)
            nc.sync.dma_start(bo_sb[:], bo[:])

            wq_sb = wsb.tile([P, NFC * OLOC], F32R, tag="wq")
            wk_sb = wsb.tile([P, NFC * OLOC], F32R, tag="wk")
            wv_sb = wsb.tile([P, NFC * OLOC], F32R, tag="wv")
            wo_sb = constp.tile([P, NFC * OLOC], F32R)
            for t_sb, t_dr in ((wq_sb, wq), (wk_sb, wk), (wv_sb, wv), (wo_sb, wo)):
                for fc in range(NFC):
                    nc.sync.dma_start(
                        t_sb[:, fc * OLOC:(fc + 1) * OLOC],
                        t_dr[fc * P:(fc + 1) * P, :],
                    )

            # ---- persistent activations
            # qT/kT: per o-block (2) x seq-block (4), [128, 512] (o on partitions)
            qT = [[qkp.tile([P, SBK], F32R, name=f"qT{ob}{sb}", tag=f"q{ob}{sb}")
                   for sb in range(NSB)] for ob in range(2)]
            kT = [[qkp.tile([P, SBK], F32R, name=f"kT{ob}{sb}", tag=f"k{ob}{sb}")
                   for sb in range(NSB)] for ob in range(2)]
            # v natural per key-chunk: [128, 4*65]; head h at cols 65h..65h+63,
            # ones column at 65h+64 (gives softmax denominator in AV row 64).
            vt = [vtp.tile([P, HLOC * (HD + 1)], F32R, name=f"vt{kc}", tag=f"v{kc}")
                  for kc in range(NKC)]
            for kc in range(NKC):
                v3 = vt[kc].rearrange("p (h x) -> p h x", h=HLOC)
                nc.sync.dma_start(v3[:, :, HD:HD + 1], ones_in[:, 64:68])
            # ctxT (normalized) per o-block x q-chunk: [128, 512]
            ctxq = [[ctxp_pool.tile([P, SBK], F32R, name=f"ctxq{ob}{qb}",
                                    tag=f"c{ob}{qb}") for qb in range(NSB)]
                    for ob in range(2)]

            # ================= phase A: q/k/v projections =================
            with (
                tc.tile_pool(name="xc", bufs=12) as xcp,
                tc.tile_pool(name="psA", bufs=4, space="PSUM") as psA,
                tc.tile_pool(name="psV", bufs=2, space="PSUM") as psV,
            ):
                for sb in range(NSB):
                    xch = []
                    for fc in range(NFC):
                        t = xcp.tile([P, SBK], F32R, tag="x")
                        nc.sync.dma_start(
                            t[:], xt[fc * P:(fc + 1) * P, sb * SBK:(sb + 1) * SBK])
                        xch.append(t)
                    for (name, w_sb, dst) in (("q", wq_sb, qT), ("k", wk_sb, kT)):
                        for ob in range(2):
                            ps = psA.tile([P, SBK], F32, tag="pqk")
                            for fc in range(NFC):
                                lhsT = w_sb[:, fc * OLOC + ob * P:
                                            fc * OLOC + (ob + 1) * P]
                                nc.tensor.matmul(ps[:], lhsT, xch[fc][:],
                                                 start=(fc == 0), stop=(fc == NFC - 1))
                            nc.vector.tensor_copy(dst[ob][sb][:], ps[:])
                    for sc in range(4):
                        kc = 4 * sb + sc
                        ps = psV.tile([P, OLOC], F32, tag="pv")
                        for fc in range(NFC):
                            lhsT = xch[fc][:, sc * P:(sc + 1) * P]
                            nc.tensor.matmul(ps[:], lhsT,
                                             wv_sb[:, fc * OLOC:(fc + 1) * OLOC],
                                             start=(fc == 0), stop=(fc == NFC - 1))
                        v3 = vt[kc].rearrange("p (h x) -> p h x", h=HLOC)
                        p3 = ps.rearrange("p (h x) -> p h x", h=HLOC)
                        nc.vector.tensor_copy(v3[:, :, 0:HD], p3[:])

            # ========== phase B+C: attention, chunked AllGather, out-proj ==========
            cin_q = [dramp.tile([OLOC, SBK], F32R, name=f"cin{qb}", tag=f"cin{qb}")
                     for qb in range(NSB)]
            cout_q = [dramp.tile([HLOC * OLOC, SBK], F32R, name=f"cout{qb}",
                                 tag=f"cout{qb}") for qb in range(NSB)]
            with (
                tc.tile_pool(name="exp", bufs=6) as expp,
                tc.tile_pool(name="sm", bufs=4) as smp,
                tc.tile_pool(name="gth", bufs=1) as gthp,
                tc.tile_pool(name="osb", bufs=4) as osbp,
                tc.tile_pool(name="psS", bufs=3, space="PSUM") as psS,
                tc.tile_pool(name="psC", bufs=2, space="PSUM") as psC,
                tc.tile_pool(name="psB", bufs=1, space="PSUM") as psB,
                tc.tile_pool(name="psO", bufs=2, space="PSUM") as psO,
            ):
                ones_l = smp.tile([1, HD], F32R, name="ones_l", tag="ones")
                nc.sync.dma_start(ones_l[:], ones_in[0:1, 0:HD])
                for qb in range(NSB):
                    nkb = 4 * qb + 4
                    # head h's denominator parked at partition 32h (alignment)
                    den = smp.tile([P, SBK], F32, name=f"den{qb}", tag="den")
                    rec = smp.tile([P, SBK], F32, name=f"rec{qb}", tag="rec")
                    ctx_ps_h = []
                    for ob in range(2):
                        for j in range(2):
                            h = 2 * ob + j
                            ctx_ps = psC.tile([HD + 1, SBK], F32, tag="ctx")
                            for kb in range(nkb):
                                t = kb - 4 * qb
                                off = 128 * t if t > 0 else 0
                                sp = psS.tile([P, SBK], F32, tag="sc")
                                nc.tensor.matmul(
                                    sp[:, off:],
                                    kT[ob][kb // 4][j * HD:(j + 1) * HD,
                                                    (kb % 4) * P:(kb % 4 + 1) * P],
                                    qT[ob][qb][j * HD:(j + 1) * HD, off:],
                                    start=True, stop=True,
                                    tile_position=(j * HD, 0),
                                )
                                et = expp.tile([P, SBK], F32R, tag="e")
                                nc.scalar.activation(et[:, off:], sp[:, off:],
                                                     Exp, scale=0.125)
                                if t >= 0:  # diagonal block: zero out k > q
                                    nc.vector.tensor_tensor(
                                        et[:, off:], et[:, off:],
                                        masks_sb[:, t * SBK + off:(t + 1) * SBK],
                                        mybir.AluOpType.mult,
                                    )
                                nc.tensor.matmul(
                                    ctx_ps[:, off:],
                                    vt[kb][:, h * (HD + 1):(h + 1) * (HD + 1)],
                                    et[:, off:],
                                    start=(kb == 0), stop=(kb == nkb - 1),
                                )
                            # stash denominator row (partition HD -> partition h)
                            nc.vector.tensor_copy(den[32 * h:32 * h + 1, :],
                                                  ctx_ps[HD:HD + 1, :])
                            cu = smp.tile([HD, SBK], F32, name=f"cu{qb}{h}",
                                          tag=f"cu{h}")
                            nc.scalar.copy(cu[:], ctx_ps[0:HD, :])
                            ctx_ps_h.append(cu)
                    # batched reciprocal for the 4 heads of this q-chunk
                    nc.vector.reciprocal(rec[:], den[:])
                    for ob in range(2):
                        for j in range(2):
                            h = 2 * ob + j
                            rcr = smp.tile([1, SBK], F32R, tag="rcr")
                            nc.vector.tensor_copy(rcr[:], rec[32 * h:32 * h + 1, :])
                            bc_ps = psB.tile([HD, SBK], F32, tag="bcp")
                            nc.tensor.matmul(bc_ps[:], ones_l[:], rcr[:],
                                             start=True, stop=True)
                            bc = smp.tile([HD, SBK], F32, tag="bc")
                            nc.vector.tensor_copy(bc[:], bc_ps[:])
                            nc.vector.tensor_tensor(
                                ctxq[ob][qb][j * HD:(j + 1) * HD, :],
                                ctx_ps_h[h][:], bc[:],
                                mybir.AluOpType.mult,
                            )
                    # ---- AllGather this q-chunk across the 4 cores of the batch
                    nc.gpsimd.dma_start(cin_q[qb][0:P, :], ctxq[0][qb][:])
                    nc.gpsimd.dma_start(cin_q[qb][P:OLOC, :], ctxq[1][qb][:])
                    nc.gpsimd.collective_compute(
                        "AllGather",
                        mybir.AluOpType.bypass,
                        replica_groups=[[0, 1, 2, 3], [4, 5, 6, 7]],
                        ins=[cin_q[qb].opt()],
                        outs=[cout_q[qb].opt()],
                    )
                    g = [gthp.tile([P, SBK], F32R, name=f"g{qb}{oc}",
                                   tag=f"g{oc}") for oc in range(NFC)]
                    for oc in range(NFC):
                        nc.gpsimd.dma_start(g[oc][:],
                                            cout_q[qb][oc * P:(oc + 1) * P, :])
                    for cb in range(2):
                        ps = psO.tile([P, SBK], F32, tag="po")
                        for oc in range(NFC):
                            lhsT = wo_sb[:, oc * OLOC + cb * P:
                                         oc * OLOC + (cb + 1) * P]
                            nc.tensor.matmul(ps[:], lhsT, g[oc][:],
                                             start=(oc == 0), stop=(oc == NFC - 1))
                        ot = osbp.tile([P, SBK], F32, tag="ot")
                        nc.scalar.activation(ot[:], ps[:], Ident,
                                             bias=bo_sb[:, cb:cb + 1], scale=1.0)
                        nc.sync.dma_start(
                            outT[cb * P:(cb + 1) * P, qb * SBK:(qb + 1) * SBK],
                            ot[:])

    _legalize_waits(nc)
    return nc
def _get_nc():
    if "nc" not in _CACHE:
        _CACHE["nc"] = _build_nc()
    return _CACHE["nc"]


LAST_RESULTS = None  # BassKernelResults of the most recent run (for profiling)


def kernel(x, Wq, Wk, Wv, Wo, bo):
    global LAST_RESULTS
    x = np.ascontiguousarray(np.asarray(x, dtype=np.float32))
    Wq = np.asarray(Wq, dtype=np.float32)
    Wk = np.asarray(Wk, dtype=np.float32)
    Wv = np.asarray(Wv, dtype=np.float32)
    Wo = np.asarray(Wo, dtype=np.float32)
    bo = np.asarray(bo, dtype=np.float32)

    # causal masks for the 4 diagonal 128x512 blocks: valid iff qi >= ki + 128*t
    ki = np.arange(P)[:, None]
    qi = np.arange(SBK)[None, :]
    masks = np.concatenate(
        [(qi >= ki + P * t).astype(np.float32) for t in range(4)], axis=1)

    in_maps = []
    for c in range(NCORES):
        b, g = divmod(c, HLOC)
        sl = slice(g * OLOC, (g + 1) * OLOC)
        in_maps.append({
            "xt": np.ascontiguousarray(x[b].T),
            "wq": np.ascontiguousarray(Wq[sl, :].T),
            "wk": np.ascontiguousarray(Wk[sl, :].T),
            "wv": np.ascontiguousarray(Wv[sl, :].T),
            "wo": np.ascontiguousarray(Wo[sl, :].T),
            "bo": np.ascontiguousarray(bo[sl].reshape(2, P).T),
            "masks": masks,
            "ones_in": np.ones((P, 68), dtype=np.float32),
        })

    nc = _get_nc()
    LAST_RESULTS = run_bass_kernel_spmd(nc, in_maps, core_ids=list(range(NCORES)))

    out = np.empty((B, S, D), dtype=np.float32)
    for c in range(NCORES):
        b, g = divmod(c, HLOC)
        out[b, :, g * OLOC:(g + 1) * OLOC] = LAST_RESULTS.results[c]["outT"].T
    return out
